# revision 12
# baseline (speedup 1.0000x reference)
"""Trainium2 Bass kernel for an encoder block (B=8, S=1024, D=768, H=12, F=3072).

Sharding: data-parallel over batch - 8 batch elements onto 8 NeuronCores, no
collectives. Each core runs the full encoder block on its [S, D] slice.

v2: fp8(e4m3) DoubleRow matmuls with residual-compensated W1/W2, mask-compacted
keys (host drops masked-out key positions; ~half the attention work), host-side
pre-transposed/pre-cast inputs, LN via scale-invariance (no unscale passes),
rstd = Exp(-0.5*Ln(var+eps)) to stay in the exp activation-table set.

Scales: weights x1024 (hi+lo fp8 pair accumulates in one PSUM group),
pT x16 (scores: psum/2048 -> exp bias -ln4 gives T/4; cancels in softmax),
vaug x64 -> ctx8 = 64*ctx, Wo psum = 65536*attn_out, xpb = 65536*(x+bo),
h1s = 1024*h1 matches FFN2 psum scale; LN is scale-invariant so no unscaling.
"""

import numpy as np
import ml_dtypes

import concourse.bass as bass
import concourse.tile as tile
from concourse import bacc
from concourse import mybir
from concourse.bass_utils import run_bass_kernel_spmd
from concourse.masks import make_identity

B, S, D, H, F = 8, 1024, 768, 12, 3072
DK = 64
P = 128
QT = S // P          # 8 query tiles
DT = D // P          # 6 d tiles
FT = F // P          # 24 f tiles
NPAIR = H // 2       # 6 head pairs
EPS = 1e-5
LN4 = 1.3862943611198906

f32 = mybir.dt.float32
f8 = mybir.dt.float8e4
bf16 = mybir.dt.bfloat16
AF = mybir.ActivationFunctionType
DR = mybir.MatmulPerfMode.DoubleRow
e4 = ml_dtypes.float8_e4m3


def q8(a, scale=1.0):
    return np.clip(np.asarray(a, np.float32) * scale, -240.0, 240.0).astype(e4)


def build_bass(KT):
    """KT = number of 128-wide compacted key tiles (<= 8)."""
    KK = KT * P
    nc = bacc.Bacc()

    xT8_d = nc.dram_tensor("xT8", [P, DT, S], f8, kind="ExternalInput")
    xk8T_d = nc.dram_tensor("xk8T", [P, DT, KK], f8, kind="ExternalInput")
    xpb_d = nc.dram_tensor("xpb", [S, D], f32, kind="ExternalInput")
    maskc_d = nc.dram_tensor("maskc", [KK], f32, kind="ExternalInput")
    maskcs_d = nc.dram_tensor("maskcs", [KK], f32, kind="ExternalInput")
    wq8_d = nc.dram_tensor("wq8", [P, DT, D], f8, kind="ExternalInput")
    wo8_d = nc.dram_tensor("wo8", [P, DT, D], f8, kind="ExternalInput")
    w18h_d = nc.dram_tensor("w18h", [P, DT, F], f8, kind="ExternalInput")
    w18l_d = nc.dram_tensor("w18l", [P, DT, F], f8, kind="ExternalInput")
    w28h_d = nc.dram_tensor("w28h", [P, FT, D], f8, kind="ExternalInput")
    w28l_d = nc.dram_tensor("w28l", [P, FT, D], f8, kind="ExternalInput")
    bqc16_d = nc.dram_tensor("bqc16", [D], f32, kind="ExternalInput")
    bq8r_d = nc.dram_tensor("bq8r", [D], f8, kind="ExternalInput")
    b12f8r_d = nc.dram_tensor("b12f8r", [D], f8, kind="ExternalInput")
    bf1f_d = nc.dram_tensor("bf1f", [F], f32, kind="ExternalInput")
    g1s_d = nc.dram_tensor("g1s", [D], f32, kind="ExternalInput")
    g2v_d = nc.dram_tensor("g2v", [D], f32, kind="ExternalInput")
    b2v_d = nc.dram_tensor("b2v", [D], f32, kind="ExternalInput")
    y_d = nc.dram_tensor("y", [S, D], f32, kind="ExternalOutput")

    with tile.TileContext(nc) as tc:
        _emit(tc, KT, xT8_d, xk8T_d, xpb_d, maskc_d, maskcs_d, wq8_d, wo8_d,
              w18h_d, w18l_d, w28h_d, w28l_d, bqc16_d, bq8r_d, b12f8r_d,
              bf1f_d, g1s_d, g2v_d, b2v_d, y_d)
    nc.compile()
    return nc


def _bcast(d):
    """DMA access pattern: [N] dram -> [128, N] partition-broadcast."""
    return bass.AP(tensor=d, offset=0, ap=[[0, P], [1, d.shape[0]]])


def _emit(tc, KT, xT8_d, xk8T_d, xpb_d, maskc_d, maskcs_d, wq8_d, wo8_d,
          w18h_d, w18l_d, w28h_d, w28l_d, bqc16_d, bq8r_d, b12f8r_d,
          bf1f_d, g1s_d, g2v_d, b2v_d, y_d):
    nc = tc.nc
    KK = KT * P
    from contextlib import ExitStack

    with ExitStack() as ctx:
        singles = ctx.enter_context(tc.tile_pool(name="singles", bufs=1, side="left"))

        ident = singles.tile([P, P], bf16, tag="ident")
        make_identity(nc, ident)
        zero_t = singles.tile([P, 1], f32, tag="zero")
        nc.vector.memset(zero_t, 0.0)
        nl4_t = singles.tile([P, 1], f32, tag="nl4")
        nc.vector.memset(nl4_t, -LN4)
        eps1_t = singles.tile([P, 1], f32, tag="eps1")
        nc.vector.memset(eps1_t, EPS * 65536.0 * 65536.0)
        eps2_t = singles.tile([P, 1], f32, tag="eps2")
        nc.vector.memset(eps2_t, EPS * 1024.0 * 1024.0)
        ones8 = singles.tile([1, P], f8, tag="ones8")
        nc.vector.memset(ones8, 1.0)

        maskcol = singles.tile([P, KT], f32, tag="maskcol")
        nc.sync.dma_start(maskcol, maskc_d[:].rearrange("(t p) -> p t", p=P))
        maskscol = singles.tile([P, KT], f32, tag="maskscol")
        nc.sync.dma_start(maskscol, maskcs_d[:].rearrange("(t p) -> p t", p=P))
        bqc16col = singles.tile([P, DT], f32, tag="bqc16col")
        nc.sync.dma_start(bqc16col, bqc16_d[:].rearrange("(t p) -> p t", p=P))
        bf1col = singles.tile([P, FT], f32, tag="bf1col")
        nc.sync.dma_start(bf1col, bf1f_d[:].rearrange("(t p) -> p t", p=P))
        bq8row = singles.tile([1, D], f8, tag="bq8row")
        nc.sync.dma_start(bq8row, bq8r_d[:].rearrange("(o n) -> o n", o=1))
        b12f8row = singles.tile([1, D], f8, tag="b12f8row")
        nc.sync.dma_start(b12f8row, b12f8r_d[:].rearrange("(o n) -> o n", o=1))
        g1sb = singles.tile([P, D], f32, tag="g1sb")
        nc.sync.dma_start(g1sb, _bcast(g1s_d))
        g2b = singles.tile([P, D], f32, tag="g2b")
        nc.sync.dma_start(g2b, _bcast(g2v_d))
        b2b = singles.tile([P, D], f32, tag="b2b")
        nc.sync.dma_start(b2b, _bcast(b2v_d))

        # ---- persistent SBUF tensors; left-side stack allocated in
        # reverse-death order (LIFO release): h1p > w1p > wop > z1p > attp
        # > projp ----
        h1p = tc.alloc_tile_pool(name="h1p", bufs=1, side="left")
        h1s = h1p.tile([P, QT, D], f32, tag="h1s", name="h1s")
        h1T8 = h1p.tile([P, DT, S], f8, tag="h1T8", name="h1T8")

        w1p = tc.alloc_tile_pool(name="w1p", bufs=1, side="left")
        w18h = w1p.tile([P, DT, F], f8, tag="w18h", name="w18h")
        w18l = w1p.tile([P, DT, F], f8, tag="w18l", name="w18l")
        nc.sync.dma_start(w18h, w18h_d[:])
        nc.sync.dma_start(w18l, w18l_d[:])

        wop = tc.alloc_tile_pool(name="wop", bufs=1, side="left")
        wo8 = wop.tile([P, DT, D], f8, tag="wo8", name="wo8")
        nc.sync.dma_start(wo8, wo8_d[:])

        z1p = tc.alloc_tile_pool(name="z1p", bufs=1, side="left")

        attp = tc.alloc_tile_pool(name="attp", bufs=1, side="left")
        pq8s = attp.tile([32, H, 2, S], f8, tag="pq8s", name="pq8s")
        pk8s = attp.tile([32, H, 2, KK], f8, tag="pk8s", name="pk8s")
        vaug8 = attp.tile([P, KT, H, 80], f8, tag="vaug8", name="vaug8")

        projp = tc.alloc_tile_pool(name="projp", bufs=1, side="left")
        xT8 = projp.tile([P, DT, S], f8, tag="xT8", name="xT8")
        xk8T = projp.tile([P, DT, KK], f8, tag="xk8T", name="xk8T")
        wq8 = projp.tile([P, DT, D], f8, tag="wq8", name="wq8")
        nc.sync.dma_start(xT8, xT8_d[:])
        nc.sync.dma_start(xk8T, xk8T_d[:])
        nc.sync.dma_start(wq8, wq8_d[:])

        ctxp = tc.alloc_tile_pool(name="ctxp", bufs=1, side="right")
        ctxT8 = ctxp.tile([P, DT, S], f8, tag="ctxT8", name="ctxT8")

        # ================= Phase A: projections =================
        with tc.tile_pool(name="ptmp", bufs=1) as ptmp, \
             tc.tile_pool(name="psA1", bufs=2, space="PSUM") as psA1, \
             tc.tile_pool(name="psA2", bufs=1, space="PSUM") as psA2, \
             tc.tile_pool(name="psA3", bufs=2, space="PSUM") as psA3:
            pT8t = ptmp.tile([P, DT, S], f8, tag="pT8t")
            pTk8t = ptmp.tile([P, DT, KK], f8, tag="pTk8t")

            for do in range(DT):
                # q-side p^T tile [do]: psum = 1024*(Wq^T x^T) per 512-col chunk
                for qc in range(2):
                    ps = psA1.tile([P, 512], f32, tag="pj")
                    for j in range(3):
                        nc.tensor.matmul(
                            ps,
                            wq8[:, 2 * j : 2 * j + 2, do * P : (do + 1) * P],
                            xT8[:, 2 * j : 2 * j + 2, qc * 512 : (qc + 1) * 512],
                            start=(j == 0), stop=(j == 2), perf_mode=DR,
                        )
                    nc.vector.tensor_scalar(
                        pT8t[:, do, qc * 512 : (qc + 1) * 512], ps,
                        0.015625, bqc16col[:, do : do + 1],
                        op0=mybir.AluOpType.mult, op1=mybir.AluOpType.add,
                    )
                # k-side p^T tile [do] over compacted keys
                psk = psA2.tile([P, KK], f32, tag="pjk")
                for kc0 in range(0, KK, 512):
                    kw = min(512, KK - kc0)
                    for j in range(3):
                        nc.tensor.matmul(
                            psk[:, kc0 : kc0 + kw],
                            wq8[:, 2 * j : 2 * j + 2, do * P : (do + 1) * P],
                            xk8T[:, 2 * j : 2 * j + 2, kc0 : kc0 + kw],
                            start=(j == 0), stop=(j == 2), perf_mode=DR,
                        )
                nc.vector.tensor_scalar(
                    pTk8t[:, do, :], psk, 0.015625, bqc16col[:, do : do + 1],
                    op0=mybir.AluOpType.mult, op1=mybir.AluOpType.add,
                )
                # fold into scores layouts [32, h, sub, cols]
                for hh in range(2):
                    h = 2 * do + hh
                    for sub in range(2):
                        p0 = hh * 64 + sub * 32
                        nc.sync.dma_start(pq8s[:, h, sub, :],
                                          pT8t[p0 : p0 + 32, do, :])
                        nc.sync.dma_start(pk8s[:, h, sub, :],
                                          pTk8t[p0 : p0 + 32, do, :])

            # p rows at compacted keys -> vaug (masked, x64)
            for kt in range(KT):
                ps = psA3.tile([P, D], f32, tag="pr")
                for oc, osz in ((0, 512), (512, 256)):
                    for j in range(3):
                        nc.tensor.matmul(
                            ps[:, oc : oc + osz],
                            xk8T[:, 2 * j : 2 * j + 2, kt * P : (kt + 1) * P],
                            wq8[:, 2 * j : 2 * j + 2, oc : oc + osz],
                            start=(j == 0), stop=False, perf_mode=DR,
                        )
                    nc.tensor.matmul(ps[:, oc : oc + osz], ones8,
                                     bq8row[:, oc : oc + osz],
                                     start=False, stop=True)
                nc.vector.tensor_scalar_mul(
                    vaug8[:, kt, :, 0:DK],
                    ps.rearrange("p (h d) -> p h d", h=H),
                    maskscol[:, kt : kt + 1],
                )
                nc.vector.tensor_copy(
                    vaug8[:, kt, :, DK : DK + 1],
                    maskcol[:, kt : kt + 1, None].to_broadcast((P, H, 1)),
                )

        projp.release()

        # ================= Phase B: attention =================
        # jh outer so ctxT8[:, :, jh*512:] completes early; Wo for the first
        # half interleaves with the second jh pass.
        def emit_wo(qt, psW, pCt):
            xpbt = pCt.tile([P, D], f32, tag="xpb")
            nc.sync.dma_start(xpbt, xpb_d[qt * P : (qt + 1) * P, :])
            ps = psW.tile([P, D], f32, tag="wo")
            for oc, osz in ((0, 512), (512, 256)):
                for j in range(3):
                    nc.tensor.matmul(
                        ps[:, oc : oc + osz],
                        ctxT8[:, 2 * j : 2 * j + 2, qt * P : (qt + 1) * P],
                        wo8[:, 2 * j : 2 * j + 2, oc : oc + osz],
                        start=(j == 0), stop=(j == 2), perf_mode=DR,
                    )
            z1 = z1p.tile([P, D], f32, tag=f"z1_{qt}", name=f"z1_{qt}")
            nc.vector.tensor_add(z1, ps, xpbt)
            st = pCt.tile([P, 3, 6], f32, tag="bnst")
            for sg in range(3):
                nc.vector.bn_stats(st[:, sg, :], z1[:, sg * 256 : (sg + 1) * 256])
            mv = z1p.tile([P, 2], f32, tag=f"mv1_{qt}", name=f"mv1_{qt}")
            nc.vector.bn_aggr(mv, st)
            return z1, mv

        NKP = KT // 2          # full DoubleRow k-tile pairs
        KODD = KT % 2 == 1

        with tc.tile_pool(name="attnz", bufs=3) as pZ, \
             tc.tile_pool(name="pCtmp", bufs=3) as pCt, \
             tc.tile_pool(name="psumW", bufs=1, space="PSUM") as psW:

            z1mv = [None] * QT

            def emit_scores(pair, jh, pB, psS):
                tblk = pB.tile([P, KT, 2, 512], f8, tag="T")
                for it in range(KT):
                    ps = psS.tile([P, 2, 512], f32, tag="sc")
                    for hh in range(2):
                        h = 2 * pair + hh
                        nc.tensor.matmul(
                            ps[:, hh, :],
                            pk8s[:, h, :, it * P : (it + 1) * P],
                            pq8s[:, h, :, jh * 512 : (jh + 1) * 512],
                            start=True, stop=True, perf_mode=DR,
                        )
                    nc.scalar.activation(tblk[:, it, :, :], ps, AF.Exp,
                                         bias=nl4_t, scale=1.0 / 2048.0)
                return tblk

            def emit_ctx(pair, jh, tblk, psC):
                for hh in range(2):
                    h = 2 * pair + hh
                    cps = psC.tile([DK + 1, 512], f32, tag="cx")
                    for t in range(NKP):
                        nc.tensor.matmul(
                            cps,
                            vaug8[:, 2 * t : 2 * t + 2, h, 0 : DK + 1],
                            tblk[:, 2 * t : 2 * t + 2, hh, :],
                            start=(t == 0), stop=(t == NKP - 1 and not KODD),
                            perf_mode=DR,
                        )
                    if KODD:
                        nc.tensor.matmul(
                            cps, vaug8[:, KT - 1, h, 0 : DK + 1],
                            tblk[:, KT - 1, hh, :],
                            start=(NKP == 0), stop=True,
                        )
                    zrow = pZ.tile([1, 512], f32, tag="zrow")
                    nc.vector.reciprocal(zrow, cps[DK : DK + 1, :])
                    invb = pZ.tile([DK, 512], f32, tag="invb")
                    nc.gpsimd.partition_broadcast(invb, zrow)
                    nc.vector.tensor_mul(
                        ctxT8[(hh * DK) : (hh + 1) * DK, pair,
                              jh * 512 : (jh + 1) * 512],
                        cps[0:DK, :], invb,
                    )

            with tc.tile_pool(name="attnT", bufs=2) as pB, \
                 tc.tile_pool(name="psumS", bufs=2, space="PSUM") as psS, \
                 tc.tile_pool(name="psumC", bufs=2, space="PSUM") as psC:
                # jh = 0
                prev = None
                for pair in range(NPAIR):
                    tb = emit_scores(pair, 0, pB, psS)
                    if prev is not None:
                        emit_ctx(pair - 1, 0, prev, psC)
                    prev = tb
                emit_ctx(NPAIR - 1, 0, prev, psC)
                # jh = 1 with Wo[qt0-3] interleaved
                prev = None
                for pair in range(NPAIR):
                    tb = emit_scores(pair, 1, pB, psS)
                    if prev is not None:
                        emit_ctx(pair - 1, 1, prev, psC)
                    prev = tb
                    if pair < 4:
                        z1mv[pair] = emit_wo(pair, psW, pCt)
                emit_ctx(NPAIR - 1, 1, prev, psC)

            # ---- post-attention: Wo qt4-7 + LN1 tails + transposes ----
            with tc.tile_pool(name="ptrp", bufs=2, space="PSUM") as psT, \
                 tc.tile_pool(name="tbp", bufs=2) as tbp:
                for qt in range(4, QT):
                    z1mv[qt] = emit_wo(qt, psW, pCt)
                for qt in range(QT):
                    z1, mv = z1mv[qt]
                    lnv = pZ.tile([P, 1], f32, tag="lnv")
                    nc.scalar.activation(lnv, mv[:, 1:2], AF.Ln,
                                         bias=eps1_t, scale=1.0)
                    rstd = pZ.tile([P, 1], f32, tag="rstd")
                    nc.scalar.activation(rstd, lnv, AF.Exp,
                                         bias=zero_t, scale=-0.5)
                    t_b = tbp.tile([P, D], bf16, tag="tb")
                    nc.vector.tensor_scalar(
                        t_b, z1, mv[:, 0:1], rstd,
                        op0=mybir.AluOpType.subtract, op1=mybir.AluOpType.mult,
                    )
                    # residual flavor: h1s = 1024*t*g1 (b1 folded into FFN2 bias)
                    nc.gpsimd.tensor_mul(h1s[:, qt, :], t_b, g1sb)
                    for di in range(DT):
                        tps = psT.tile([P, P], bf16, tag="tph")
                        nc.tensor.transpose(
                            tps, t_b[:, di * P : (di + 1) * P], ident)
                        nc.vector.tensor_copy(
                            h1T8[:, di, qt * P : (qt + 1) * P], tps)

        attp.release()
        z1p.release()
        wop.release()
        ctxp.release()

        # ================= Phase D: FFN =================
        gp = tc.alloc_tile_pool(name="gp", bufs=1, side="right")
        gT8 = gp.tile([P, FT, S], f8, tag="gT8", name="gT8")

        w2p = tc.alloc_tile_pool(name="w2p", bufs=1, side="right")
        w28h = w2p.tile([P, FT, D], f8, tag="w28h", name="w28h")
        w28l = w2p.tile([P, FT, D], f8, tag="w28l", name="w28l")
        nc.sync.dma_start(w28h, w28h_d[:])
        nc.sync.dma_start(w28l, w28l_d[:])

        with tc.tile_pool(name="pDt", bufs=3) as pDt, \
             tc.tile_pool(name="psF1", bufs=2, space="PSUM") as psF1, \
             tc.tile_pool(name="psF2", bufs=2, space="PSUM") as psF2:

            def emit_ffn1(ft, qc):
                ps = psF1.tile([P, 512], f32, tag="f1")
                for w18 in (w18h, w18l):
                    for j in range(3):
                        nc.tensor.matmul(
                            ps,
                            w18[:, 2 * j : 2 * j + 2, ft * P : (ft + 1) * P],
                            h1T8[:, 2 * j : 2 * j + 2, qc * 512 : (qc + 1) * 512],
                            start=(w18 is w18h and j == 0),
                            stop=(w18 is w18l and j == 2),
                            perf_mode=DR,
                        )
                nc.scalar.activation(
                    gT8[:, ft, qc * 512 : (qc + 1) * 512], ps, AF.Gelu,
                    bias=bf1col[:, ft : ft + 1], scale=1.0 / 1024.0,
                )

            def emit_ffn2(qt):
                ps = psF2.tile([P, D], f32, tag="f2")
                for oc, osz in ((0, 512), (512, 256)):
                    for w28 in (w28h, w28l):
                        for t in range(FT // 2):
                            nc.tensor.matmul(
                                ps[:, oc : oc + osz],
                                gT8[:, 2 * t : 2 * t + 2, qt * P : (qt + 1) * P],
                                w28[:, 2 * t : 2 * t + 2, oc : oc + osz],
                                start=(w28 is w28h and t == 0), stop=False,
                                perf_mode=DR,
                            )
                    nc.tensor.matmul(ps[:, oc : oc + osz], ones8,
                                     b12f8row[:, oc : oc + osz],
                                     start=False, stop=True)
                z2 = pDt.tile([P, D], f32, tag="z2")
                nc.vector.tensor_add(z2, ps, h1s[:, qt, :])
                st = pDt.tile([P, 3, 6], f32, tag="bnst2")
                for sg in range(3):
                    nc.vector.bn_stats(st[:, sg, :], z2[:, sg * 256 : (sg + 1) * 256])
                mv = pDt.tile([P, 2], f32, tag="mv2", name=f"mv2_{qt}")
                nc.vector.bn_aggr(mv, st)
                return z2, mv

            def emit_out(qt, z2, mv):
                lnv = pDt.tile([P, 1], f32, tag="lnv2")
                nc.scalar.activation(lnv, mv[:, 1:2], AF.Ln,
                                     bias=eps2_t, scale=1.0)
                rstd = pDt.tile([P, 1], f32, tag="rstd2")
                nc.scalar.activation(rstd, lnv, AF.Exp, bias=zero_t, scale=-0.5)
                t2 = pDt.tile([P, D], f32, tag="t2")
                nc.vector.tensor_scalar(
                    t2, z2, mv[:, 0:1], rstd,
                    op0=mybir.AluOpType.subtract, op1=mybir.AluOpType.mult,
                )
                nc.gpsimd.tensor_mul(t2, t2, g2b)
                out_t = pDt.tile([P, D], f32, tag="outt")
                nc.vector.tensor_add(out_t, t2, b2b)
                nc.sync.dma_start(y_d[qt * P : (qt + 1) * P, :], out_t)

            for ft in range(FT):
                emit_ffn1(ft, 0)
            for ft in range(FT):
                emit_ffn1(ft, 1)
            z2s = []
            for qt in range(0, 4):
                z2s.append(emit_ffn2(qt))
            # LN2 rstds batched after gelus on the ACT queue
            for qt in range(0, 4):
                emit_out(qt, *z2s[qt])
            z2s2 = []
            for qt in range(4, QT):
                z2s2.append(emit_ffn2(qt))
            for qt in range(4, QT):
                emit_out(qt, *z2s2[qt - 4])

        w2p.release()
        gp.release()
        w1p.release()
        h1p.release()


_BASS_CACHE = {}


def _get_bass(KT):
    if KT not in _BASS_CACHE:
        _BASS_CACHE[KT] = build_bass(KT)
    return _BASS_CACHE[KT]


def _prep_shared(inputs):
    w = {k: np.asarray(inputs[k], np.float32)
         for k in ("Wq", "bq", "Wo", "bo", "g1", "b1", "W1", "bf1", "W2",
                   "bf2", "g2", "b2")}
    W1f = w["g1"][:, None] * w["W1"]
    w18h = q8(1024.0 * W1f)
    w18l = q8(1024.0 * W1f - w18h.astype(np.float32))
    w28h = q8(1024.0 * w["W2"])
    w28l = q8(1024.0 * w["W2"] - w28h.astype(np.float32))

    def tile_kd(a, nt):  # [nt*128, N] -> [128, nt, N]
        return np.ascontiguousarray(
            a.reshape(nt, P, a.shape[1]).transpose(1, 0, 2))

    shared = {
        "wq8": tile_kd(q8(1024.0 * w["Wq"]), DT),
        "wo8": tile_kd(q8(1024.0 * w["Wo"]), DT),
        "w18h": tile_kd(w18h, DT),
        "w18l": tile_kd(w18l, DT),
        "w28h": tile_kd(w28h, FT),
        "w28l": tile_kd(w28l, FT),
        "bqc16": 16.0 * w["bq"],
        "bq8r": q8(1024.0 * w["bq"]),
        "b12f8r": q8(1024.0 * (w["b1"] + w["bf2"])),
        "bf1f": w["b1"] @ W1f + w["bf1"],
        "g1s": 1024.0 * w["g1"],
        "g2v": w["g2"],
        "b2v": w["b2"],
    }
    bo = w["bo"]
    return shared, bo


def prepare(inputs):
    """Host-side prep: returns (nc, in_maps) for run_bass_kernel_spmd."""
    x = np.asarray(inputs["x"], np.float32)
    mask = (np.asarray(inputs["attn_mask"]) != 0)
    shared, bo = _prep_shared(inputs)

    counts = mask.sum(axis=1)
    KT = max(1, int(np.ceil(counts.max() / P)))
    KK = KT * P
    nc = _get_bass(KT)

    in_maps = []
    for b in range(B):
        xb = x[b]
        idx = np.nonzero(mask[b])[0]
        nk = len(idx)
        idx_pad = np.zeros(KK, np.int64)
        idx_pad[:nk] = idx
        maskc = np.zeros(KK, np.float32)
        maskc[:nk] = 1.0

        x8T = q8(xb.T)                       # [768, 1024]
        xk8 = x8T[:, idx_pad]                # [768, KK] compacted keys
        m = {
            "xT8": np.ascontiguousarray(
                x8T.reshape(DT, P, S).transpose(1, 0, 2)),
            "xk8T": np.ascontiguousarray(
                xk8.reshape(DT, P, KK).transpose(1, 0, 2)),
            "xpb": 65536.0 * (xb + bo[None, :]),
            "maskc": maskc,
            "maskcs": maskc * 0.0625,
        }
        m.update(shared)
        in_maps.append(m)
    return nc, in_maps


def kernel(**inputs):
    nc, in_maps = prepare(inputs)
    res = run_bass_kernel_spmd(nc, in_maps, core_ids=list(range(B)))
    return np.stack([res.results[b]["y"] for b in range(B)], axis=0)


if __name__ == "__main__":
    nc = build_bass(5)
    print("bass build ok")


# revision 34
# speedup vs baseline: 33.5603x; 33.5603x over previous
"""Trainium2 Bass kernel for an encoder block (B=8, S=1024, D=768, H=12, F=3072).

Sharding: data-parallel over batch - 8 batch elements onto 8 NeuronCores, no
collectives. Each core runs the full encoder block on its [S, D] slice.

v2: fp8(e4m3) DoubleRow matmuls with residual-compensated W1/W2, mask-compacted
keys (host drops masked-out key positions; ~half the attention work), host-side
pre-transposed/pre-cast inputs, LN via scale-invariance (no unscale passes),
rstd = Exp(-0.5*Ln(var+eps)) to stay in the exp activation-table set.

Scales: weights x1024 (hi+lo fp8 pair accumulates in one PSUM group),
pT x16 (scores: psum/2048 -> exp bias -ln4 gives T/4; cancels in softmax),
vaug x64 -> ctx8 = 64*ctx, Wo psum = 65536*attn_out, xpb = 65536*(x+bo),
h1s = 1024*h1 matches FFN2 psum scale; LN is scale-invariant so no unscaling.
"""

import numpy as np
import ml_dtypes

import concourse.bass as bass
import concourse.tile as tile
from concourse import bacc
from concourse import mybir
from concourse.bass_utils import run_bass_kernel_spmd
from concourse.masks import make_identity

B, S, D, H, F = 8, 1024, 768, 12, 3072
DK = 64
P = 128
QT = S // P          # 8 query tiles
DT = D // P          # 6 d tiles
FT = F // P          # 24 f tiles
NPAIR = H // 2       # 6 head pairs
EPS = 1e-5
LN4 = 1.3862943611198906

f32 = mybir.dt.float32
f8 = mybir.dt.float8e4
bf16 = mybir.dt.bfloat16
AF = mybir.ActivationFunctionType
DR = mybir.MatmulPerfMode.DoubleRow
e4 = ml_dtypes.float8_e4m3


def q8(a, scale=1.0):
    return np.clip(np.asarray(a, np.float32) * scale, -240.0, 240.0).astype(e4)


def build_bass(KT):
    """KT = number of 128-wide compacted key tiles (<= 8)."""
    KK = KT * P
    nc = bacc.Bacc()

    xT8_d = nc.dram_tensor("xT8", [P, DT, S], f8, kind="ExternalInput")
    xk8T_d = nc.dram_tensor("xk8T", [P, DT, KK], f8, kind="ExternalInput")
    xpb_d = nc.dram_tensor("xpb", [S, D], f32, kind="ExternalInput")
    maskc_d = nc.dram_tensor("maskc", [KK], f32, kind="ExternalInput")
    maskcs_d = nc.dram_tensor("maskcs", [KK], f32, kind="ExternalInput")
    wq8_d = nc.dram_tensor("wq8", [P, DT, D], f8, kind="ExternalInput")
    wo8_d = nc.dram_tensor("wo8", [P, DT, D], f8, kind="ExternalInput")
    w18h_d = nc.dram_tensor("w18h", [P, DT, F], f8, kind="ExternalInput")
    w18l_d = nc.dram_tensor("w18l", [P, DT, F], f8, kind="ExternalInput")
    w28h_d = nc.dram_tensor("w28h", [P, FT, D], f8, kind="ExternalInput")
    w28l_d = nc.dram_tensor("w28l", [P, FT, D], f8, kind="ExternalInput")
    bqc16_d = nc.dram_tensor("bqc16", [D], f32, kind="ExternalInput")
    bq8r_d = nc.dram_tensor("bq8r", [D], f8, kind="ExternalInput")
    b12f8r_d = nc.dram_tensor("b12f8r", [D], f8, kind="ExternalInput")
    bf1f_d = nc.dram_tensor("bf1f", [F], f32, kind="ExternalInput")
    g1s_d = nc.dram_tensor("g1s", [D], f32, kind="ExternalInput")
    g2v_d = nc.dram_tensor("g2v", [D], f32, kind="ExternalInput")
    b2v_d = nc.dram_tensor("b2v", [D], f32, kind="ExternalInput")
    y_d = nc.dram_tensor("y", [S, D], f32, kind="ExternalOutput")

    with tile.TileContext(nc) as tc:
        _emit(tc, KT, xT8_d, xk8T_d, xpb_d, maskc_d, maskcs_d, wq8_d, wo8_d,
              w18h_d, w18l_d, w28h_d, w28l_d, bqc16_d, bq8r_d, b12f8r_d,
              bf1f_d, g1s_d, g2v_d, b2v_d, y_d)
    nc.compile()
    return nc


def _bcast(d):
    """DMA access pattern: [N] dram -> [128, N] partition-broadcast."""
    return bass.AP(tensor=d, offset=0, ap=[[0, P], [1, d.shape[0]]])


def _emit(tc, KT, xT8_d, xk8T_d, xpb_d, maskc_d, maskcs_d, wq8_d, wo8_d,
          w18h_d, w18l_d, w28h_d, w28l_d, bqc16_d, bq8r_d, b12f8r_d,
          bf1f_d, g1s_d, g2v_d, b2v_d, y_d):
    nc = tc.nc
    KK = KT * P
    from contextlib import ExitStack

    with ExitStack() as ctx:
        singles = ctx.enter_context(tc.tile_pool(name="singles", bufs=1, side="left"))

        ident = singles.tile([P, P], bf16, tag="ident")
        make_identity(nc, ident)
        nl4_t = singles.tile([P, 1], f32, tag="nl4")
        nc.vector.memset(nl4_t, -LN4)
        ones8 = singles.tile([1, P], f8, tag="ones8")
        nc.vector.memset(ones8, 1.0)

        maskcol = singles.tile([P, KT], f32, tag="maskcol")
        maskscol = singles.tile([P, KT], f32, tag="maskscol")
        bqc16col = singles.tile([P, DT], f32, tag="bqc16col")
        bf1col = singles.tile([P, FT], f32, tag="bf1col")
        bq8row = singles.tile([1, D], f8, tag="bq8row")
        b12f8row = singles.tile([1, D], f8, tag="b12f8row")
        g1sb = singles.tile([P, D], f32, tag="g1sb")

        # ---- persistent SBUF tensors; left-side stack allocated in
        # reverse-death order (LIFO release): h1p > w1p > wop > z1p > attp
        # > projp ----
        h1p = tc.alloc_tile_pool(name="h1p", bufs=1, side="left")
        h1s = h1p.tile([P, QT, D], bf16, tag="h1s", name="h1s")
        h1T8 = h1p.tile([P, DT, S], f8, tag="h1T8", name="h1T8")
        gT8 = h1p.tile([P, FT, S], f8, tag="gT8", name="gT8")

        w1p = tc.alloc_tile_pool(name="w1p", bufs=1, side="left")
        w18h = w1p.tile([P, DT, F], f8, tag="w18h", name="w18h")
        w18l = w1p.tile([P, DT, F], f8, tag="w18l", name="w18l")

        wop = tc.alloc_tile_pool(name="wop", bufs=1, side="left")
        wo8 = wop.tile([P, DT, D], f8, tag="wo8", name="wo8")

        z1p = tc.alloc_tile_pool(name="z1p", bufs=1, side="left")

        attp = tc.alloc_tile_pool(name="attp", bufs=1, side="left")
        pq8s = attp.tile([32, H, 2, S], f8, tag="pq8s", name="pq8s")
        pk8s = attp.tile([32, H, 2, KK], f8, tag="pk8s", name="pk8s")
        vaug8 = attp.tile([P, KT, H, 80], f8, tag="vaug8", name="vaug8")

        projp = tc.alloc_tile_pool(name="projp", bufs=1, side="left")
        xT8 = projp.tile([P, DT, S], f8, tag="xT8", name="xT8")
        xk8T = projp.tile([P, DT, KK], f8, tag="xk8T", name="xk8T")
        wq8 = projp.tile([P, DT, D], f8, tag="wq8", name="wq8")
        nc.sync.dma_start(xk8T, xk8T_d[:])
        nc.sync.dma_start(wq8, wq8_d[:])
        nc.sync.dma_start(xT8, xT8_d[:])
        nc.sync.dma_start(bqc16col, bqc16_d[:].rearrange("(t p) -> p t", p=P))
        nc.sync.dma_start(maskcol, maskc_d[:].rearrange("(t p) -> p t", p=P))
        nc.sync.dma_start(maskscol, maskcs_d[:].rearrange("(t p) -> p t", p=P))
        nc.sync.dma_start(bq8row, bq8r_d[:].rearrange("(o n) -> o n", o=1))
        nc.sync.dma_start(g1sb, _bcast(g1s_d))
        nc.sync.dma_start(bf1col, bf1f_d[:].rearrange("(t p) -> p t", p=P))
        nc.sync.dma_start(b12f8row, b12f8r_d[:].rearrange("(o n) -> o n", o=1))

        ctxp = tc.alloc_tile_pool(name="ctxp", bufs=1, side="right")
        ctxT8 = ctxp.tile([P, DT, S], f8, tag="ctxT8", name="ctxT8")

        # ======== Phase A (projections) interleaved with attention ========
        A_ = mybir.AluOpType
        u32 = mybir.dt.uint32

        def emit_rsqrt(eng, pool, var_ap, eps, tag):
            """rstd = 1/sqrt(var + eps): magic seed + 2 Newton steps. ``eng``
            does the arithmetic; the u32 bit-ops always run on DVE (gpsimd
            cannot lower them)."""
            a = pool.tile([P, 1], f32, tag=tag + "a")
            y = pool.tile([P, 1], f32, tag=tag + "y")
            t = pool.tile([P, 1], f32, tag=tag + "t")
            eng.tensor_scalar(a, var_ap, eps, None, op0=A_.add)
            nc.vector.tensor_scalar(y.bitcast(u32), a.bitcast(u32), 1, None,
                                    op0=A_.logical_shift_right)
            nc.vector.tensor_copy(t, y.bitcast(u32))
            eng.tensor_scalar(t, t, 1597463007.0, -1.0,
                              op0=A_.subtract, op1=A_.mult)
            nc.vector.tensor_copy(y.bitcast(u32), t)
            for _ in range(2):
                eng.tensor_mul(t, y, y)
                eng.tensor_mul(t, t, a)
                eng.tensor_scalar(t, t, -0.5, 1.5, op0=A_.mult, op1=A_.add)
                eng.tensor_mul(y, y, t)
            return y

        NKP = KT // 2          # full DoubleRow k-tile pairs
        KODD = KT % 2 == 1
        EPS1 = EPS * 65536.0 * 65536.0
        EPS2 = EPS * 1024.0 * 1024.0

        with tc.tile_pool(name="attnz", bufs=2) as pZ, \
             tc.tile_pool(name="pCtmp", bufs=2) as pCt, \
             tc.tile_pool(name="rsq", bufs=2) as pRs, \
             tc.tile_pool(name="ptmp", bufs=2) as ptmp:

            tbs = [None] * QT

            def emit_wo(qt, psW):
                """Wo matmuls + residual add; returns z1 for the LN1 tail."""
                xpbt = pCt.tile([P, D], f32, tag="xpb")
                nc.sync.dma_start(xpbt, xpb_d[qt * P : (qt + 1) * P, :])
                ps = psW.tile([P, D], f32, tag="wo")
                for oc, osz in ((0, 512), (512, 256)):
                    for j in range(3):
                        nc.tensor.matmul(
                            ps[:, oc : oc + osz],
                            ctxT8[:, 2 * j : 2 * j + 2, qt * P : (qt + 1) * P],
                            wo8[:, 2 * j : 2 * j + 2, oc : oc + osz],
                            start=(j == 0), stop=(j == 2), perf_mode=DR,
                        )
                z1 = z1p.tile([P, D], bf16, tag=f"z1_{qt}", name=f"z1_{qt}")
                nc.vector.tensor_add(z1, ps, xpbt)
                return z1

            def emit_ln1(qt, z1):
                """LN1 stats + Newton rstd + t_b + h1s (no ACT involvement)."""
                st = pCt.tile([P, 3, 6], f32, tag="bnst")
                for sg in range(3):
                    nc.vector.bn_stats(st[:, sg, :], z1[:, sg * 256 : (sg + 1) * 256])
                mv = pCt.tile([P, 2], f32, tag="mv")
                nc.vector.bn_aggr(mv, st)
                rstd = emit_rsqrt(nc.vector, pRs, mv[:, 1:2], EPS1, "r1")
                t_b = z1p.tile([P, D], bf16, tag=f"tb_{qt}", name=f"tb_{qt}")
                nc.vector.tensor_scalar(
                    t_b, z1, mv[:, 0:1], rstd,
                    op0=A_.subtract, op1=A_.mult,
                )
                # residual flavor: 1024*t*g1 (b1 folded into FFN2 bias row)
                nc.gpsimd.tensor_mul(h1s[:, qt, :], t_b, g1sb)
                tbs[qt] = t_b

            def emit_scores(pair, jh, pB, psS):
                tblk = pB.tile([P, KT, 2, 512], f8, tag="T")
                for it in range(KT):
                    ps = psS.tile([P, 2, 512], f32, tag="sc")
                    for hh in range(2):
                        h = 2 * pair + hh
                        nc.tensor.matmul(
                            ps[:, hh, :],
                            pk8s[:, h, :, it * P : (it + 1) * P],
                            pq8s[:, h, :, jh * 512 : (jh + 1) * 512],
                            start=True, stop=True, perf_mode=DR,
                        )
                    nc.scalar.activation(tblk[:, it, :, :], ps, AF.Exp,
                                         bias=nl4_t, scale=1.0 / 2048.0)
                return tblk

            def emit_ctx(pair, jh, tblk, psC):
                for hh in range(2):
                    h = 2 * pair + hh
                    cps = psC.tile([DK + 1, 512], f32, tag="cx")
                    for t in range(NKP):
                        nc.tensor.matmul(
                            cps,
                            vaug8[:, 2 * t : 2 * t + 2, h, 0 : DK + 1],
                            tblk[:, 2 * t : 2 * t + 2, hh, :],
                            start=(t == 0), stop=(t == NKP - 1 and not KODD),
                            perf_mode=DR,
                        )
                    if KODD:
                        nc.tensor.matmul(
                            cps, vaug8[:, KT - 1, h, 0 : DK + 1],
                            tblk[:, KT - 1, hh, :],
                            start=(NKP == 0), stop=True,
                        )
                    zrow = pZ.tile([1, 512], f32, tag="zrow")
                    nc.vector.reciprocal(zrow, cps[DK : DK + 1, :])
                    invb = pZ.tile([DK, 512], f32, tag="invb")
                    nc.gpsimd.partition_broadcast(invb, zrow)
                    nc.vector.tensor_mul(
                        ctxT8[(hh * DK) : (hh + 1) * DK, pair,
                              jh * 512 : (jh + 1) * 512],
                        cps[0:DK, :], invb,
                    )

            # --- p rows at compacted keys -> vaug (masked, x64), first so the
            # ctx pipeline can start as soon as the first scores tile lands.
            with tc.tile_pool(name="psA3", bufs=2, space="PSUM") as psA3:
                for kt in range(KT):
                    ps = psA3.tile([P, D], f32, tag="pr")
                    for oc, osz in ((0, 512), (512, 256)):
                        for j in range(3):
                            nc.tensor.matmul(
                                ps[:, oc : oc + osz],
                                xk8T[:, 2 * j : 2 * j + 2, kt * P : (kt + 1) * P],
                                wq8[:, 2 * j : 2 * j + 2, oc : oc + osz],
                                start=(j == 0), stop=False, perf_mode=DR,
                            )
                        nc.tensor.matmul(ps[:, oc : oc + osz], ones8,
                                         bq8row[:, oc : oc + osz],
                                         start=False, stop=True)
                    nc.vector.tensor_scalar_mul(
                        vaug8[:, kt, :, 0:DK],
                        ps.rearrange("p (h d) -> p h d", h=H),
                        maskscol[:, kt : kt + 1],
                    )
                    nc.vector.tensor_copy(
                        vaug8[:, kt, :, DK : DK + 1],
                        maskcol[:, kt : kt + 1, None].to_broadcast((P, H, 1)),
                    )

            def emit_proj(do, psA1):
                ptq = ptmp.tile([P, S], f8, tag="ptq")
                ptk = ptmp.tile([P, KK], f8, tag="ptk")
                # q-side p^T tile [do]
                for qc in range(2):
                    ps = psA1.tile([P, 512], f32, tag="pj")
                    for j in range(3):
                        nc.tensor.matmul(
                            ps,
                            wq8[:, 2 * j : 2 * j + 2, do * P : (do + 1) * P],
                            xT8[:, 2 * j : 2 * j + 2, qc * 512 : (qc + 1) * 512],
                            start=(j == 0), stop=(j == 2), perf_mode=DR,
                        )
                    nc.vector.tensor_scalar(
                        ptq[:, qc * 512 : (qc + 1) * 512], ps,
                        0.015625, bqc16col[:, do : do + 1],
                        op0=A_.mult, op1=A_.add,
                    )
                # k-side p^T tile [do] over compacted keys
                for kc0 in range(0, KK, 512):
                    kw = min(512, KK - kc0)
                    ps = psA1.tile([P, 512], f32, tag="pj")
                    for j in range(3):
                        nc.tensor.matmul(
                            ps[:, :kw],
                            wq8[:, 2 * j : 2 * j + 2, do * P : (do + 1) * P],
                            xk8T[:, 2 * j : 2 * j + 2, kc0 : kc0 + kw],
                            start=(j == 0), stop=(j == 2), perf_mode=DR,
                        )
                    nc.vector.tensor_scalar(
                        ptk[:, kc0 : kc0 + kw], ps[:, :kw],
                        0.015625, bqc16col[:, do : do + 1],
                        op0=A_.mult, op1=A_.add,
                    )
                # fold into scores layouts [32, h, sub, cols]
                for hh in range(2):
                    h = 2 * do + hh
                    for sub in range(2):
                        p0 = hh * 64 + sub * 32
                        nc.sync.dma_start(pq8s[:, h, sub, :],
                                          ptq[p0 : p0 + 32, :])
                        nc.sync.dma_start(pk8s[:, h, sub, :],
                                          ptk[p0 : p0 + 32, :])

            with tc.tile_pool(name="attnT", bufs=2) as pB, \
                 tc.tile_pool(name="psumS", bufs=2, space="PSUM") as psS, \
                 tc.tile_pool(name="psumC", bufs=2, space="PSUM") as psC:
                tbq = {}
                with tc.tile_pool(name="psA1", bufs=2, space="PSUM") as psA1:
                    for do in range(DT):
                        emit_proj(do, psA1)
                        # big weight loads, chunked so fold DMAs never queue
                        # behind a multi-us transfer
                        nc.sync.dma_start(w18h[:, do, :], w18h_d[:, do, :])
                        nc.sync.dma_start(w18l[:, do, :], w18l_d[:, do, :])
                        if do >= 4:
                            nc.sync.dma_start(
                                wo8[:, 2 * (do - 4) : 2 * (do - 3), :],
                                wo8_d[:, 2 * (do - 4) : 2 * (do - 3), :])
                        if do == 5:
                            nc.sync.dma_start(wo8[:, 4:6, :], wo8_d[:, 4:6, :])
                        if do >= 1:
                            tbq[do - 1] = emit_scores(do - 1, 0, pB, psS)
                        if do >= 2:
                            emit_ctx(do - 2, 0, tbq.pop(do - 2), psC)
                tbq[NPAIR - 1] = emit_scores(NPAIR - 1, 0, pB, psS)
                emit_ctx(NPAIR - 2, 0, tbq.pop(NPAIR - 2), psC)
                emit_ctx(NPAIR - 1, 0, tbq.pop(NPAIR - 1), psC)
                # jh = 1 with Wo[qt0-3] + LN1 interleaved; Wo qt4-7 right
                # after the last ctx so psW can close with psS/psC
                z1s_ = [None] * QT
                with tc.tile_pool(name="psumW", bufs=1, space="PSUM") as psW:
                    prev = None
                    for pair in range(NPAIR):
                        tb = emit_scores(pair, 1, pB, psS)
                        if prev is not None:
                            emit_ctx(pair - 1, 1, prev, psC)
                        prev = tb
                        if pair >= 2:
                            z1s_[pair - 2] = emit_wo(pair - 2, psW)
                    emit_ctx(NPAIR - 1, 1, prev, psC)
                    for qt in range(2, QT):
                        z1s_[qt] = emit_wo(qt, psW)

            # ---- post-attention: transposes qt0-3 -> FFN1 qc0 -> Wo qt4-7
            # (+ LN1 chains) -> transposes qt4-7 -> FFN1 qc1 ----
            def emit_transp(qt, psT):
                for base, nw in ((0, 4), (4, 2)):
                    tps = psT.tile([P, 4, P], bf16, tag="tph")
                    for k in range(nw):
                        nc.tensor.transpose(
                            tps[:, k, :],
                            tbs[qt][:, (base + k) * P : (base + k + 1) * P],
                            ident)
                    nc.vector.tensor_copy(
                        h1T8[:, base : base + nw, qt * P : (qt + 1) * P],
                        tps[:, 0:nw, :])

            with tc.tile_pool(name="ptrp", bufs=2, space="PSUM") as psT, \
                 tc.tile_pool(name="psF1", bufs=2, space="PSUM") as psF1:

                def emit_ffn1(ft, qc):
                    ps = psF1.tile([P, 512], f32, tag="f1")
                    for w18 in (w18h, w18l):
                        for j in range(3):
                            nc.tensor.matmul(
                                ps,
                                w18[:, 2 * j : 2 * j + 2, ft * P : (ft + 1) * P],
                                h1T8[:, 2 * j : 2 * j + 2,
                                     qc * 512 : (qc + 1) * 512],
                                start=(w18 is w18h and j == 0),
                                stop=(w18 is w18l and j == 2),
                                perf_mode=DR,
                            )
                    nc.scalar.activation(
                        gT8[:, ft, qc * 512 : (qc + 1) * 512], ps, AF.Gelu,
                        bias=bf1col[:, ft : ft + 1], scale=1.0 / 1024.0,
                    )

                for qt in range(4):
                    emit_ln1(qt, z1s_[qt])
                for qt in range(4):
                    emit_transp(qt, psT)
                for ft in range(FT):
                    emit_ffn1(ft, 0)
                for qt in range(4, QT):
                    emit_ln1(qt, z1s_[qt])
                for qt in range(4, QT):
                    emit_transp(qt, psT)
                for ft in range(FT):
                    emit_ffn1(ft, 1)

        projp.release()
        attp.release()
        z1p.release()
        wop.release()
        ctxp.release()

        # ================= Phase D: FFN2 + LN2 =================
        w2p = tc.alloc_tile_pool(name="w2p", bufs=1, side="right")
        w28h = w2p.tile([P, FT, D], f8, tag="w28h", name="w28h")
        w28l = w2p.tile([P, FT, D], f8, tag="w28l", name="w28l")
        g2b = w2p.tile([P, D], f32, tag="g2b", name="g2b")
        b2b = w2p.tile([P, D], f32, tag="b2b", name="b2b")
        nc.sync.dma_start(g2b, _bcast(g2v_d))
        nc.sync.dma_start(b2b, _bcast(b2v_d))
        nc.sync.dma_start(w28h, w28h_d[:])
        nc.sync.dma_start(w28l, w28l_d[:])

        with tc.tile_pool(name="pDt", bufs=2) as pDt, \
             tc.tile_pool(name="rsq2", bufs=2) as pR2, \
             tc.tile_pool(name="psF2", bufs=2, space="PSUM") as psF2:

            def emit_ffn2(qt):
                ps = psF2.tile([P, D], f32, tag="f2")
                z2 = pDt.tile([P, D], f32, tag=f"z2_{qt % 4}")
                st = pDt.tile([P, 3, 6], f32, tag="bnst2")
                for oc, osz in ((0, 512), (512, 256)):
                    for w28 in (w28h, w28l):
                        for t in range(FT // 2):
                            nc.tensor.matmul(
                                ps[:, oc : oc + osz],
                                gT8[:, 2 * t : 2 * t + 2, qt * P : (qt + 1) * P],
                                w28[:, 2 * t : 2 * t + 2, oc : oc + osz],
                                start=(w28 is w28h and t == 0), stop=False,
                                perf_mode=DR,
                            )
                    nc.tensor.matmul(ps[:, oc : oc + osz], ones8,
                                     b12f8row[:, oc : oc + osz],
                                     start=False, stop=True)
                    # z2 + stats per chunk so LN2 overlaps the second chunk
                    nc.vector.tensor_add(z2[:, oc : oc + osz], ps[:, oc : oc + osz],
                                         h1s[:, qt, oc : oc + osz])
                    for sg in range(oc // 256, (oc + osz) // 256):
                        nc.vector.bn_stats(
                            st[:, sg, :], z2[:, sg * 256 : (sg + 1) * 256])
                mv = pDt.tile([P, 2], f32, tag=f"mv2_{qt % 4}")
                nc.vector.bn_aggr(mv, st)
                return z2, mv

            def emit_out(qt, z2, mv, eng):
                rstd = emit_rsqrt(nc.vector, pR2, mv[:, 1:2], EPS2, "r2")
                t2 = pDt.tile([P, D], f32, tag="t2")
                nc.vector.tensor_scalar(
                    t2, z2, mv[:, 0:1], rstd,
                    op0=A_.subtract, op1=A_.mult,
                )
                eng.tensor_mul(t2, t2, g2b)
                out_t = pDt.tile([P, D], f32, tag="outt")
                eng.tensor_add(out_t, t2, b2b)
                nc.sync.dma_start(y_d[qt * P : (qt + 1) * P, :], out_t)

            for gi, group in enumerate((range(0, 4), range(4, QT))):
                z2s = [emit_ffn2(qt) for qt in group]
                for qt, zm in zip(group, z2s):
                    eng = nc.vector if (gi == 1 and qt >= 6) else nc.gpsimd
                    emit_out(qt, *zm, eng)

        w2p.release()
        w1p.release()
        h1p.release()


_BASS_CACHE = {}


def _get_bass(KT):
    if KT not in _BASS_CACHE:
        _BASS_CACHE[KT] = build_bass(KT)
    return _BASS_CACHE[KT]


def _prep_shared(inputs):
    w = {k: np.asarray(inputs[k], np.float32)
         for k in ("Wq", "bq", "Wo", "bo", "g1", "b1", "W1", "bf1", "W2",
                   "bf2", "g2", "b2")}
    W1f = w["g1"][:, None] * w["W1"]
    w18h = q8(1024.0 * W1f)
    w18l = q8(1024.0 * W1f - w18h.astype(np.float32))
    w28h = q8(1024.0 * w["W2"])
    w28l = q8(1024.0 * w["W2"] - w28h.astype(np.float32))

    def tile_kd(a, nt):  # [nt*128, N] -> [128, nt, N]
        return np.ascontiguousarray(
            a.reshape(nt, P, a.shape[1]).transpose(1, 0, 2))

    shared = {
        "wq8": tile_kd(q8(1024.0 * w["Wq"]), DT),
        "wo8": tile_kd(q8(1024.0 * w["Wo"]), DT),
        "w18h": tile_kd(w18h, DT),
        "w18l": tile_kd(w18l, DT),
        "w28h": tile_kd(w28h, FT),
        "w28l": tile_kd(w28l, FT),
        "bqc16": 16.0 * w["bq"],
        "bq8r": q8(1024.0 * w["bq"]),
        "b12f8r": q8(1024.0 * (w["b1"] + w["bf2"])),
        "bf1f": w["b1"] @ W1f + w["bf1"],
        "g1s": 1024.0 * w["g1"],
        "g2v": w["g2"],
        "b2v": w["b2"],
    }
    bo = w["bo"]
    return shared, bo


def prepare(inputs):
    """Host-side prep: returns (nc, in_maps) for run_bass_kernel_spmd."""
    x = np.asarray(inputs["x"], np.float32)
    mask = (np.asarray(inputs["attn_mask"]) != 0)
    shared, bo = _prep_shared(inputs)

    counts = mask.sum(axis=1)
    KT = max(1, int(np.ceil(counts.max() / P)))
    KK = KT * P
    nc = _get_bass(KT)

    in_maps = []
    for b in range(B):
        xb = x[b]
        idx = np.nonzero(mask[b])[0]
        nk = len(idx)
        idx_pad = np.zeros(KK, np.int64)
        idx_pad[:nk] = idx
        maskc = np.zeros(KK, np.float32)
        maskc[:nk] = 1.0

        x8T = q8(xb.T)                       # [768, 1024]
        xk8 = x8T[:, idx_pad]                # [768, KK] compacted keys
        m = {
            "xT8": np.ascontiguousarray(
                x8T.reshape(DT, P, S).transpose(1, 0, 2)),
            "xk8T": np.ascontiguousarray(
                xk8.reshape(DT, P, KK).transpose(1, 0, 2)),
            "xpb": 65536.0 * (xb + bo[None, :]),
            "maskc": maskc,
            "maskcs": maskc * 0.0625,
        }
        m.update(shared)
        in_maps.append(m)
    return nc, in_maps


def kernel(**inputs):
    nc, in_maps = prepare(inputs)
    res = run_bass_kernel_spmd(nc, in_maps, core_ids=list(range(B)))
    return np.stack([res.results[b]["y"] for b in range(B)], axis=0)


if __name__ == "__main__":
    nc = build_bass(5)
    print("bass build ok")
